# revision 12
# baseline (speedup 1.0000x reference)
"""Trainium2 Bass kernel for fused GQA attention block (B=2, L=2048, D=2048,
H=16 q-heads, KV=4 kv-heads, HD=64, causal, QK-RMSNorm + RoPE).

Sharding (8 cores): core c -> batch b = c // 4, head-group g = c % 4
(query heads 4g..4g+3, kv head g). Each core computes its 4 heads'
attention and a partial output projection (256 of 1024 e-channels);
host sums the 4 partials per batch.
"""

import numpy as np

import concourse.bass as bass
import concourse.mybir as mybir
import concourse.tile as tile
from concourse import bacc
from concourse import bass_utils
from concourse.masks import make_identity

F32 = mybir.dt.float32
F32R = mybir.dt.float32r
AF = mybir.ActivationFunctionType
ALU = mybir.AluOpType

B, L, D = 2, 2048, 2048
H, KV, HD = 16, 4, 64
EPS = 1e-6
ROPE_BASE = 10000.0
N_CORES = 8
GQ = H // KV          # 4 query heads per core
LT = L // 128         # 16 l-tiles
DT = D // 128         # 16 d-tiles (contraction tiles for qkv proj)
TQ = 512              # q-chunk width for attention
NQC = L // TQ         # 4 q-chunks
NKB = L // 128        # 16 k-blocks
EW = (GQ + 2) * HD    # 384 qkv channels per core
EO = GQ * HD          # 256 output channels per core


def _classify_mask(mask):
    """Per (kb, qt) block: 'skip' | 'full' | pattern index into mixed list.

    Patterns are transposed slices maskT[k0:k0+128, q0:q0+TQ]."""
    kinds = {}
    patterns = []
    pat_ids = {}
    for qt in range(NQC):
        for kb in range(NKB):
            sub = mask[qt * TQ:(qt + 1) * TQ, kb * 128:(kb + 1) * 128]
            if np.all(sub <= -1e8):
                kinds[(kb, qt)] = "skip"
            elif np.all(sub == 0.0):
                kinds[(kb, qt)] = "full"
            else:
                pt = np.ascontiguousarray(sub.T.astype(np.float32))
                key = pt.tobytes()
                if key not in pat_ids:
                    pat_ids[key] = len(patterns)
                    patterns.append(pt)
                kinds[(kb, qt)] = pat_ids[key]
    return kinds, patterns


def _build_program(kinds, n_mixed):
    nc = bacc.Bacc("TRN2", target_bir_lowering=False, debug=False,
                   enable_asserts=False, num_devices=N_CORES)

    # DRAM I/O (per core). Host pre-tiles everything into DMA-friendly layouts.
    xT = nc.dram_tensor("xT", [LT, D, 128], F32, kind="ExternalInput").ap()
    wqkT = nc.dram_tensor("wqkT", [D, EW], F32, kind="ExternalInput").ap()
    woT = nc.dram_tensor("woT", [EO, D], F32, kind="ExternalInput").ap()
    cos2 = nc.dram_tensor("cos2", [128, LT * 32], F32, kind="ExternalInput").ap()
    sin2 = nc.dram_tensor("sin2", [128, LT * 32], F32, kind="ExternalInput").ap()
    qw = nc.dram_tensor("qw", [128, GQ * HD], F32, kind="ExternalInput").ap()
    kw = nc.dram_tensor("kw", [128, HD], F32, kind="ExternalInput").ap()
    if n_mixed:
        mblk = nc.dram_tensor("mblk", [128, n_mixed * TQ], F32,
                              kind="ExternalInput").ap()
    y = nc.dram_tensor("y", [L, D], F32, kind="ExternalOutput").ap()

    with tile.TileContext(nc) as tc:
        with (
            tc.tile_pool(name="consts", bufs=1) as consts,
            tc.tile_pool(name="wpool", bufs=1) as wpool,
            tc.tile_pool(name="xcolp", bufs=3) as xcolp,
            tc.tile_pool(name="work", bufs=3) as work,
            tc.tile_pool(name="persist", bufs=1) as persist,
            tc.tile_pool(name="pp", bufs=3) as pp,
            tc.tile_pool(name="zp", bufs=4) as zp,
            tc.tile_pool(name="ps_a", bufs=2, space="PSUM") as ps_a,
            tc.tile_pool(name="ps_b", bufs=2, space="PSUM") as ps_b,
            tc.tile_pool(name="ps_sc", bufs=2, space="PSUM") as ps_sc,
        ):
            # ---- constants ----
            ident = consts.tile([128, 128], F32, tag="ident")
            make_identity(nc, ident[:])
            cos_sb = consts.tile([128, LT * 32], F32, tag="cos")
            sin_sb = consts.tile([128, LT * 32], F32, tag="sin")
            nc.sync.dma_start(cos_sb[:], cos2[:])
            nc.sync.dma_start(sin_sb[:], sin2[:])
            eps_sb = consts.tile([128, 1], F32, tag="eps")
            nc.vector.memset(eps_sb[:], EPS)
            qw_sb = consts.tile([128, GQ * HD], F32, tag="qw")
            kw_sb = consts.tile([128, HD], F32, tag="kw")
            nc.sync.dma_start(qw_sb[:], qw[:])
            nc.sync.dma_start(kw_sb[:], kw[:])
            if n_mixed:
                mb_sb = consts.tile([128, n_mixed * TQ], F32, tag="mb")
                nc.sync.dma_start(mb_sb[:], mblk[:])

            # ---- weights (fp32r rounded during SWDGE cast-DMA) ----
            wqk_sb = []
            for dt_i in range(DT):
                w = wpool.tile([128, EW], F32R, tag=f"wqk{dt_i}")
                nc.gpsimd.dma_start(w[:], wqkT[dt_i * 128:(dt_i + 1) * 128, :])
                wqk_sb.append(w)
            wo_sb = []
            for et in range(2):
                w = wpool.tile([128, D], F32R, tag=f"wo{et}")
                nc.gpsimd.dma_start(w[:], woT[et * 128:(et + 1) * 128, :])
                wo_sb.append(w)

            # ---- persistent attention operands ----
            # Q^T per head [64, L]; K^T [64, L]; V-hat [128, 65 per k-tile]
            qt_sb = [persist.tile([64, L], F32R, tag=f"qt{h}", name=f"qt{h}")
                     for h in range(GQ)]
            kt_sb = persist.tile([64, L], F32R, tag="kt")
            vt_sb = persist.tile([128, LT * (HD + 1)], F32R, tag="vt")
            # ones column of V-hat (col HD of each 65-wide group)
            ones_sb = consts.tile([128, 1], F32, tag="ones")
            nc.vector.memset(ones_sb[:], 1.0)
            for i in range(LT):
                nc.vector.tensor_copy(
                    vt_sb[:, i * (HD + 1) + HD:i * (HD + 1) + HD + 1],
                    ones_sb[:])
            aot_sb = [persist.tile([128, L], F32R, tag=f"aot{et}",
                                   name=f"aot{et}")
                      for et in range(2)]

            # ================= Phase 1: QKV + RMSNorm + RoPE =================
            for lt in range(LT):
                xcol = xcolp.tile([128, D], F32R, tag="xcol")
                nc.gpsimd.dma_start(
                    xcol[:].rearrange("p (t j) -> p t j", j=128),
                    xT[lt, :, :].rearrange("(t p) j -> p t j", p=128))
                qkv_ps = ps_a.tile([128, EW], F32, tag="mm_a")
                for dt_i in range(DT):
                    nc.tensor.matmul(
                        qkv_ps[:], xcol[:, dt_i * 128:(dt_i + 1) * 128],
                        wqk_sb[dt_i][:],
                        start=(dt_i == 0), stop=(dt_i == DT - 1))

                # RMS stats for 5 norm groups (4 q heads + 1 k head)
                sq_scr = work.tile([128, HD], F32, tag="sq_scr")
                ss = work.tile([128, 16], F32, tag="ss")
                for i in range(GQ + 1):
                    nc.scalar.activation(
                        sq_scr[:], qkv_ps[:, i * HD:(i + 1) * HD], AF.Square,
                        accum_out=ss[:, i:i + 1])
                # rstd = 1/sqrt(ss/HD + eps)
                nc.scalar.activation(ss[:, 5:5 + GQ + 1], ss[:, 0:GQ + 1],
                                     AF.Sqrt, bias=eps_sb[:], scale=1.0 / HD)
                rstd = work.tile([128, GQ + 1], F32, tag="rstd")
                nc.vector.reciprocal(rstd[:], ss[:, 5:5 + GQ + 1])

                # normalize * weight
                qn = work.tile([128, GQ * HD], F32, tag="qn")
                nc.vector.tensor_tensor(
                    qn[:].rearrange("p (h e) -> p h e", e=HD),
                    qkv_ps[:, 0:GQ * HD].rearrange("p (h e) -> p h e", e=HD),
                    rstd[:, 0:GQ, None].broadcast_to([128, GQ, HD]),
                    op=ALU.mult)
                nc.vector.tensor_tensor(qn[:], qn[:], qw_sb[:], op=ALU.mult)
                kn = work.tile([128, HD], F32, tag="kn")
                nc.vector.scalar_tensor_tensor(
                    kn[:], qkv_ps[:, GQ * HD:(GQ + 1) * HD], rstd[:, GQ:GQ + 1],
                    kw_sb[:], op0=ALU.mult, op1=ALU.mult)

                # RoPE on q (all 4 heads at once) and k
                cs = cos_sb[:, lt * 32:(lt + 1) * 32]
                sn = sin_sb[:, lt * 32:(lt + 1) * 32]
                csq = cs[:, None, :].broadcast_to([128, GQ, 32])
                snq = sn[:, None, :].broadcast_to([128, GQ, 32])
                rq = work.tile([128, GQ * HD], F32, tag="rq")
                rqv = rq[:].rearrange("p (h e) -> p h e", e=HD)
                qnv = qn[:].rearrange("p (h e) -> p h e", e=HD)
                t1 = work.tile([128, GQ * 32], F32, tag="t1")
                t1v = t1[:].rearrange("p (h e) -> p h e", e=32)
                # low half: x1*cos - x2*sin
                nc.vector.tensor_tensor(t1v, qnv[:, :, 0:32], csq, op=ALU.mult)
                nc.vector.tensor_tensor(rqv[:, :, 0:32], qnv[:, :, 32:64], snq,
                                        op=ALU.mult)
                nc.vector.tensor_tensor(rqv[:, :, 0:32], t1v,
                                        rqv[:, :, 0:32], op=ALU.subtract)
                # high half: x1*sin + x2*cos
                nc.vector.tensor_tensor(t1v, qnv[:, :, 0:32], snq, op=ALU.mult)
                nc.vector.tensor_tensor(rqv[:, :, 32:64], qnv[:, :, 32:64], csq,
                                        op=ALU.mult)
                nc.vector.tensor_tensor(rqv[:, :, 32:64], t1v,
                                        rqv[:, :, 32:64], op=ALU.add)
                rk = work.tile([128, HD], F32, tag="rk")
                t2 = work.tile([128, 32], F32, tag="t2")
                nc.vector.tensor_tensor(t2[:], kn[:, 0:32], cs, op=ALU.mult)
                nc.vector.tensor_tensor(rk[:, 0:32], kn[:, 32:64], sn, op=ALU.mult)
                nc.vector.tensor_tensor(rk[:, 0:32], t2[:], rk[:, 0:32],
                                        op=ALU.subtract)
                nc.vector.tensor_tensor(t2[:], kn[:, 0:32], sn, op=ALU.mult)
                nc.vector.tensor_tensor(rk[:, 32:64], kn[:, 32:64], cs, op=ALU.mult)
                nc.vector.tensor_tensor(rk[:, 32:64], t2[:], rk[:, 32:64],
                                        op=ALU.add)

                # transposes: q heads + k -> [64, 128] tiles
                for h in range(GQ):
                    tp = ps_b.tile([64, 128], F32, tag="mm_b")
                    nc.tensor.matmul(tp[:], rq[:, h * HD:(h + 1) * HD],
                                     ident[:], is_transpose=True)
                    nc.vector.tensor_copy(
                        qt_sb[h][:, lt * 128:(lt + 1) * 128], tp[:])
                tp = ps_b.tile([64, 128], F32, tag="mm_b")
                nc.tensor.matmul(tp[:], rk[:], ident[:], is_transpose=True)
                nc.vector.tensor_copy(kt_sb[:, lt * 128:(lt + 1) * 128], tp[:])

                # V natural layout [l(k), dd] + ones col
                nc.vector.tensor_copy(
                    vt_sb[:, lt * (HD + 1):lt * (HD + 1) + HD],
                    qkv_ps[:, (GQ + 1) * HD:(GQ + 2) * HD])

            # ================= Phase 2: attention =================
            for h in range(GQ):
                for qc in range(NQC):
                    klist = [kb for kb in range(NKB)
                             if kinds[(kb, qc)] != "skip"]
                    if not klist:
                        continue
                    av_ps = ps_b.tile([HD + 1, TQ], F32, tag="mm_b")
                    first = True
                    # chunks of 2 k-blocks share one exp call
                    for ci in range(0, len(klist), 2):
                        chunk = klist[ci:ci + 2]
                        sc_ps = ps_sc.tile([128, 1024], F32, tag="sc")
                        for j, kb in enumerate(chunk):
                            nc.tensor.matmul(
                                sc_ps[:, j * TQ:(j + 1) * TQ],
                                kt_sb[:, kb * 128:(kb + 1) * 128],
                                qt_sb[h][:, qc * TQ:(qc + 1) * TQ],
                                start=True, stop=True)
                            kind = kinds[(kb, qc)]
                            if kind != "full":
                                nc.vector.tensor_tensor(
                                    sc_ps[:, j * TQ:(j + 1) * TQ],
                                    sc_ps[:, j * TQ:(j + 1) * TQ],
                                    mb_sb[:, kind * TQ:(kind + 1) * TQ],
                                    op=ALU.add)
                        p_sb = pp.tile([128, 1024], F32R, tag="p")
                        nw = len(chunk) * TQ
                        nc.scalar.activation(p_sb[:, 0:nw], sc_ps[:, 0:nw],
                                             AF.Exp)
                        for j, kb in enumerate(chunk):
                            last = (ci + j == len(klist) - 1)
                            nc.tensor.matmul(
                                av_ps[:],
                                vt_sb[:, kb * (HD + 1):(kb + 1) * (HD + 1)],
                                p_sb[:, j * TQ:(j + 1) * TQ],
                                start=first, stop=last)
                            first = False
                    # normalize by the ones-column denominator
                    rec = work.tile([1, TQ], F32, tag="rec")
                    nc.vector.reciprocal(rec[:], av_ps[HD:HD + 1, :])
                    bcr = work.tile([64, TQ], F32, tag="bcr")
                    nc.gpsimd.partition_broadcast(bcr[:], rec[:])
                    et, sub = divmod(h, 2)
                    nc.vector.tensor_tensor(
                        aot_sb[et][sub * 64:(sub + 1) * 64,
                                   qc * TQ:(qc + 1) * TQ],
                        av_ps[0:HD, :], bcr[:], op=ALU.mult)

            # ================= Phase 3: output projection =================
            for lt in range(LT):
                for dc in range(4):
                    z_ps = ps_a.tile([128, 512], F32, tag="mm_a")
                    for et in range(2):
                        nc.tensor.matmul(
                            z_ps[:], aot_sb[et][:, lt * 128:(lt + 1) * 128],
                            wo_sb[et][:, dc * 512:(dc + 1) * 512],
                            start=(et == 0), stop=(et == 1))
                    zo = zp.tile([128, 512], F32, tag="zo")
                    nc.vector.tensor_copy(zo[:], z_ps[:])
                    nc.sync.dma_start(
                        y[lt * 128:(lt + 1) * 128, dc * 512:(dc + 1) * 512],
                        zo[:])

    nc.compile()
    return nc


_PROGRAM_CACHE = {}


def _get_program(kinds, n_mixed):
    key = (tuple(sorted(kinds.items())), n_mixed)
    if key not in _PROGRAM_CACHE:
        _PROGRAM_CACHE[key] = _build_program(kinds, n_mixed)
    return _PROGRAM_CACHE[key]


def _host_prep(x, W_qkv, W_out, q_norm_w, k_norm_w, mask):
    kinds, patterns = _classify_mask(np.asarray(mask))
    n_mixed = len(patterns)
    assert n_mixed <= 12, f"too many unique mask patterns: {n_mixed}"

    # RoPE tables, tiled [128, LT*32]: cos2[p, lt*32+j] = cos((lt*128+p)*freq_j)
    j = np.arange(0, HD, 2, dtype=np.float32)
    freqs = (ROPE_BASE ** (-j / HD)).astype(np.float32)
    pos = np.arange(L, dtype=np.float32)
    theta = pos[:, None] * freqs[None, :]
    cosf = np.cos(theta).astype(np.float32)     # [L, 32]
    sinf = np.sin(theta).astype(np.float32)
    cos2 = np.ascontiguousarray(
        cosf.reshape(LT, 128, 32).transpose(1, 0, 2).reshape(128, LT * 32))
    sin2 = np.ascontiguousarray(
        sinf.reshape(LT, 128, 32).transpose(1, 0, 2).reshape(128, LT * 32))

    scale = np.float32(HD ** -0.5)
    qwv = (np.asarray(q_norm_w, np.float32) * scale)
    qw_rep = np.tile(np.tile(qwv, GQ)[None, :], (128, 1)).astype(np.float32)
    kw_rep = np.tile(np.asarray(k_norm_w, np.float32)[None, :], (128, 1))

    if n_mixed:
        mb = np.concatenate(patterns, axis=1).astype(np.float32)  # [128, nm*TQ]
    else:
        mb = None

    in_maps = []
    for c in range(N_CORES):
        b, g = divmod(c, KV)
        xb = np.asarray(x[b], np.float32)
        xTt = np.ascontiguousarray(
            xb.reshape(LT, 128, D).transpose(0, 2, 1))      # [LT, D, 128]
        rows = np.r_[g * GQ * HD:(g + 1) * GQ * HD,
                     (H + g) * HD:(H + g + 1) * HD,
                     (H + KV + g) * HD:(H + KV + g + 1) * HD]
        wqkT = np.ascontiguousarray(np.asarray(W_qkv, np.float32)[rows].T)
        cols = np.arange(g * GQ * HD, (g + 1) * GQ * HD)
        woT = np.ascontiguousarray(np.asarray(W_out, np.float32)[:, cols].T)
        m = {"xT": xTt, "wqkT": wqkT, "woT": woT,
             "cos2": cos2, "sin2": sin2, "qw": qw_rep, "kw": kw_rep}
        if mb is not None:
            m["mblk"] = mb
        in_maps.append(m)
    return kinds, n_mixed, in_maps


def kernel(x, W_qkv, W_out, q_norm_w, k_norm_w, mask):
    kinds, n_mixed, in_maps = _host_prep(x, W_qkv, W_out, q_norm_w,
                                         k_norm_w, mask)
    nc = _get_program(kinds, n_mixed)
    res = bass_utils.run_bass_kernel_spmd(nc, in_maps,
                                          core_ids=list(range(N_CORES)))
    out = np.zeros((B, L, D), dtype=np.float32)
    for c in range(N_CORES):
        b = c // KV
        out[b] += res.results[c]["y"]
    return out
